# revision 20
# baseline (speedup 1.0000x reference)
"""Trainium2 Bass kernel for nn_AliasFreeActivation (alias-free GAN activation).

Pipeline per (n, c) plane X [64, 64]:
    y = Wdn.T @ ( sqrt(2) * lrelu_0.2( Wup.T @ (X + b) @ Wup ) ) @ Wdn
where Wup [64, 128] / Wdn [128, 64] are the upfirdn band matrices for the
separable 12-tap filter (up=2 / down=2), built on host.

Device mapping (fused matmul chain, software-pipelined stages):
    A(g): DMA x, M1 (4 MMs, K=64): t1[(e,w), h'] -> DVE copy -> t1sb
    B(g): M2 (2 row-tiled MMs K=64 N=512): u[w', (p4,h')] -> ACT Prelu -> ssb
    C(g): M3 (8 MMs K=128 N=64): v[h', (c,w'')] -> DVE copy -> vsb
          M4 (2 col-tiled MMs K=128 M=64): y -> ACT copy (1 instr / 2
          groups, Y_BATCH) -> ysb -> DMA out (4-group batches, 2KB lines)
Stages are emitted with skew (A(g+2), B(g+1), C(g)) so the cross-engine
chain M1 -> t1copy -> M2 -> Prelu -> M3 never stalls any engine.

Steady state is a three-way tie near the machine's floor for this
structure (per 8-channel group: ACT Prelu+y ~1.43us, DVE t1+v copies
~1.41us, PE 16 MMs ~1.44us): the 2304 PSUM->SBUF evac columns per group
are limited to 1 col/cycle/engine (fp32 PSUM) on exactly two engines —
GpSimd/DMA cannot touch PSUM and TRN2 matmuls cannot write bf16 PSUM.
PSUM layout (8 banks exactly): t1ps 2, ups 2x2 (double buffer), vps 1,
yps 1 (Y_BATCH pairs two groups per bank).

Sharding: pure data parallel over batch: core i gets input[i] -> [512, 64, 64].
"""

import os
import sys

for _p in ("/opt/trn_rl_repo", "/opt/pypackages"):
    if _p not in sys.path:
        sys.path.append(_p)

import numpy as np
import ml_dtypes

N_CORES = 8
B, C, H, W = 8, 512, 64, 64
GROUP = 8                 # channel planes per group
N_GROUPS = C // GROUP     # 64
DMA_BATCH = int(os.environ.get("DMAB", 4))  # groups per DMA transfer
UP_LEN = 128
NEG_SLOPE = 0.2
SQRT2 = float(2.0 ** 0.5)

def _default_cfg():
    return {
        "EVAC_T1": os.environ.get("EVAC_T1", "vector"),
        "EVAC_V": os.environ.get("EVAC_V", "vector"),
        "EVAC_Y": os.environ.get("EVAC_Y", "act"),
        "Y_COLTILE": os.environ.get("Y_COLTILE", "1") == "1",
        "M2_ROWTILE": os.environ.get("M2_ROWTILE", "1") == "1",
        "M1_ROWTILE": os.environ.get("M1_ROWTILE", "1") == "1",
        # columns of the [128, 1024] lrelu handled by ACT Prelu; the rest
        # go to DVE as a 2-op (mul to scratch, tensor_tensor max) sequence
        "PRELU_ACT_COLS": int(os.environ.get("PRELU_ACT_COLS", 1024)),
        "SKEW_B": int(os.environ.get("SKEW_B", 1)),
        "SKEW_C": int(os.environ.get("SKEW_C", 2)),
        "T1PB": int(os.environ.get("T1PB", 1)),
        "UPB": int(os.environ.get("UPB", 2)),
        "VPB": int(os.environ.get("VPB", 1)),
        "YPB": int(os.environ.get("YPB", 1)),
        "XB": int(os.environ.get("XB", 3)),
        "T1SB": int(os.environ.get("T1SB", 3)),
        "SSB": int(os.environ.get("SSB", 3)),
        "VSB": int(os.environ.get("VSB", 3)),
        "YSB": int(os.environ.get("YSB", 2)),
        "DMAB": int(os.environ.get("DMAB", 4)),
        # lrelu(u) = 0.8*relu(u) + 0.2*u: relu evac (splittable ACT/DVE)
        # + linear term as extra accumulating matmul M3b from t1sb
        "RELU_SPLIT": os.environ.get("RELU_SPLIT", "0") == "1",
        "RELU_ACT_COLS": int(os.environ.get("RELU_ACT_COLS", 640)),
        "YC_BATCH": os.environ.get("YC_BATCH", "0") == "1",
        # lrelu evac mode: "act" = ACT Prelu (+DVE 2-op tail per
        # PRELU_ACT_COLS), "gpsimd" = copy u to SBUF (split at
        # UCOPY_ACT_COLS between ACT and DVE) + lrelu on GPSIMD
        "PRELU_MODE": os.environ.get("PRELU_MODE", "act"),
        "UCOPY_ACT_COLS": int(os.environ.get("UCOPY_ACT_COLS", 1024)),
        # emit per-stage matmuls as one PSUM accumulation group (single
        # sem-inc on the last MM instead of one per MM)
        "MM_GROUP": os.environ.get("MM_GROUP", "0") == "1",
        "B_FIRST": os.environ.get("B_FIRST", "0") == "1",
        # batch the y evacuation over pairs of groups (one copy instr per
        # 2 groups, yps sized [128, 512] = one PSUM bank)
        "Y_BATCH": os.environ.get("Y_BATCH", "1") == "1",
        # ablations (timing experiments only — break correctness):
        # comma-set of {m1,t1c,m2,prelu,m3,vc,m4,yc}
        "ABL": frozenset(
            s for s in os.environ.get("ABL", "").split(",") if s),
    }

# 12-tap hann-windowed-sinc lowpass, as in the reference module
_FILT = np.array([0.0, 0.00398, -0.01884, -0.05155, 0.12443, 0.44197,
                  0.44197, 0.12443, -0.05155, -0.01884, 0.00398, 0.0],
                 dtype=np.float64)
_FILT = _FILT / _FILT.sum()

_BF16 = ml_dtypes.bfloat16

_LAST_RESULT = None   # BassKernelResults of the most recent run (for test.py)
_CACHED = None        # compiled nc cache so repeat kernel() calls skip rebuild


def _upfirdn_matrix(k, L, up, down, pad0, pad1):
    """Band matrix Wf such that y = x @ Wf applies upfirdn along an axis."""
    K = len(k)
    Ld = (L - 1) * up + 1
    n_out = (Ld + pad0 + (pad1 + up - 1) - K) // down + 1
    Wf = np.zeros((L, n_out), dtype=np.float64)
    for j in range(n_out):
        for t in range(K):
            m = j * down + t - pad0
            if 0 <= m < Ld and m % up == 0:
                Wf[m // up, j] += k[K - 1 - t]
    return Wf


def _build_consts(up_filter, down_filter, relu_split=None):
    if relu_split is None:
        relu_split = _default_cfg()["RELU_SPLIT"]
    k_up = np.asarray(up_filter, dtype=np.float64) * 2.0   # prescaled by UP
    k_dn = np.asarray(down_filter, dtype=np.float64)
    Wup = _upfirdn_matrix(k_up, 64, 2, 1, 6, 5)            # [64, 128]
    Wdn = _upfirdn_matrix(k_dn, 128, 1, 2, 5, 5)           # [128, 64]
    wup2 = np.concatenate([Wup, Wup], axis=0)              # [128, 128] vstack
    # sqrt2 (and the 0.8 relu weight in split mode) folded into M3's Wdn
    wdn3 = Wdn * SQRT2 * (1.0 - NEG_SLOPE if relu_split else 1.0)
    wdn2 = np.concatenate([Wdn, Wdn], axis=1)              # [128, 128] hstack
    # M3b: linear 0.2*sqrt2 * (Wup @ Wdn), block-diagonal over the 2 planes
    Wc = NEG_SLOPE * SQRT2 * (Wup @ Wdn)                   # [64, 64]
    wc2 = np.zeros((128, 128), dtype=np.float64)
    wc2[0:64, 0:64] = Wc
    wc2[64:128, 64:128] = Wc
    return (wup2.astype(_BF16), wdn3.astype(_BF16), wdn2.astype(_BF16),
            wc2.astype(_BF16))


def _pack_x(x_core, rowtile=None):
    if rowtile is None:
        rowtile = _default_cfg()["M1_ROWTILE"]
    if rowtile:
        # x_dev[rh*64+h, ((g*2+s)*2+e)*64+w] = x[g*8+4s+2rh+e, h, w]
        xg = x_core.reshape(N_GROUPS, 2, 2, 2, H, W)    # [g,s,rh,e,h,w]
        xd = xg.transpose(2, 4, 0, 1, 3, 5)             # [rh,h,g,s,e,w]
        return np.ascontiguousarray(
            xd.reshape(128, N_GROUPS * 4 * W).astype(_BF16))
    # x_dev[h, ((g*4+p4)*2+e)*64+w] = x[g*8+p4*2+e, h, w]
    xd = x_core.reshape(C, H, W).transpose(1, 0, 2)     # [h, c, w]
    return np.ascontiguousarray(xd.reshape(H, C * W).astype(_BF16))


def _unpack_y(y_dev):
    """[128, C*W/2] bf16 -> [C, H, W] f32.

    y_dev[ph*64 + h, (g*4 + pl)*64 + w] = y[g*8 + ph*4 + pl, h, w]
    """
    yg = y_dev.astype(np.float32).reshape(2, H, N_GROUPS, 4, W)  # [ph,h,g,pl,w]
    return np.ascontiguousarray(
        yg.transpose(2, 0, 3, 1, 4).reshape(C, H, W))


def _build_bass(n_groups=N_GROUPS, repeat=1, **overrides):
    import concourse.bacc as bacc
    import concourse.mybir as mybir
    from concourse.tile import TileContext

    cfg = _default_cfg()
    cfg.update(overrides)

    f32 = mybir.dt.float32
    bf16 = mybir.dt.bfloat16

    nc = bacc.Bacc("TRN2", target_bir_lowering=False)

    if cfg["M1_ROWTILE"]:
        x = nc.dram_tensor("x", [128, n_groups * 4 * W], bf16,
                           kind="ExternalInput")
    else:
        x = nc.dram_tensor("x", [H, n_groups * 8 * W], bf16,
                           kind="ExternalInput")
    wup2 = nc.dram_tensor("wup2", [128, 128], bf16, kind="ExternalInput")
    wdn3 = nc.dram_tensor("wdn3", [UP_LEN, W], bf16, kind="ExternalInput")
    wdn2 = nc.dram_tensor("wdn2", [128, 128], bf16, kind="ExternalInput")
    wc2 = nc.dram_tensor("wc2", [128, 128], bf16, kind="ExternalInput")
    out = nc.dram_tensor("out", [128, n_groups * 4 * W], bf16,
                         kind="ExternalOutput")

    dma_b = min(cfg["DMAB"], n_groups)
    assert n_groups % dma_b == 0

    with TileContext(nc) as tc:
        with (
            tc.tile_pool(name="consts", bufs=1) as cpool,
            tc.tile_pool(name="xt", bufs=cfg["XB"]) as xpool,
            tc.tile_pool(name="t1ps", bufs=cfg["T1PB"], space="PSUM") as t1ps_pool,
            tc.tile_pool(name="t1sb", bufs=cfg["T1SB"]) as t1sb_pool,
            tc.tile_pool(name="ups", bufs=cfg["UPB"], space="PSUM") as ups_pool,
            tc.tile_pool(name="ssb", bufs=cfg["SSB"]) as ssb_pool,
            tc.tile_pool(name="vps", bufs=cfg["VPB"], space="PSUM") as vps_pool,
            tc.tile_pool(name="vsb", bufs=cfg["VSB"]) as vsb_pool,
            tc.tile_pool(name="yps", bufs=cfg["YPB"], space="PSUM") as yps_pool,
            tc.tile_pool(name="ysb", bufs=cfg["YSB"]) as ysb_pool,
        ):
            wup2_sb = cpool.tile([128, 128], bf16)
            nc.sync.dma_start(out=wup2_sb[:], in_=wup2[:])
            wdn3_sb = cpool.tile([UP_LEN, W], bf16)
            nc.sync.dma_start(out=wdn3_sb[:], in_=wdn3[:])
            wdn2_sb = cpool.tile([128, 128], bf16)
            nc.sync.dma_start(out=wdn2_sb[:], in_=wdn2[:])
            wc2_sb = cpool.tile([128, 128], bf16)
            nc.sync.dma_start(out=wc2_sb[:], in_=wc2[:])
            alpha_sb = cpool.tile([128, 1], f32)
            nc.vector.memset(alpha_sb[:], NEG_SLOPE)

            import contextlib
            rep_ctx = (tc.For_i(0, repeat, 1) if repeat > 1
                       else contextlib.nullcontext())
            with rep_ctx:
                _pipelined_groups(nc, tc, mybir, n_groups, dma_b, cfg, locals())

    nc.compile()
    if os.environ.get("SEMOPT", "1") == "1":
        _strip_redundant_mm_incs(nc)
    return nc


def _strip_redundant_mm_incs(nc):
    """Drop per-Matmult sem-incs that no wait references.

    Tile attaches a serialized sem-inc (~26 ns EVT_SEM write) to every
    matmul; MMs complete in program order, so an inc is only load-bearing
    at positions some wait actually references. Keep incs at waited
    positions (plus the final one), remap every wait value to its rank
    among kept positions, and fix the loop reset add/sub-imm totals.
    """
    import bisect

    fn = nc.m.functions[0]
    blocks = fn.blocks

    mm_sems = set()
    for b in blocks:
        for i in b.instructions:
            si = i.sync_info
            if si and i.opcode == "Matmult":
                for u in si.on_update:
                    if u.update_mode == "sem-inc":
                        mm_sems.add(u.id)

    for s in mm_sems:
        waits = set()
        incs = []
        ok = True
        for b in blocks:
            for i in b.instructions:
                si = i.sync_info
                if not si:
                    continue
                for w in si.on_wait:
                    if w.id == s:
                        if w.wait_reg is not None or w.wait_mode != "sem-ge-imm":
                            ok = False
                        else:
                            waits.add(w.wait_value)
                for u in si.on_update:
                    if u.id == s and u.update_mode == "sem-inc":
                        if i.opcode != "Matmult" or u.update_value != 1:
                            ok = False
                        incs.append(i)
        if not ok or not incs:
            continue
        total = len(incs)
        keep = sorted(set(v for v in waits if 1 <= v <= total) | {total})

        pos = 0
        for i in incs:
            pos += 1
            if pos in keep:
                continue
            si = i.sync_info
            new_upd = [u for u in si.on_update
                       if not (u.id == s and u.update_mode == "sem-inc")]
            i.sync_info = si.__replace__(on_update=new_upd)

        for b in blocks:
            for i in b.instructions:
                si = i.sync_info
                if not si:
                    continue
                changed = False
                new_waits = []
                for w in si.on_wait:
                    if w.id == s and w.wait_value > 0:
                        nv = bisect.bisect_right(keep, w.wait_value)
                        if nv != w.wait_value:
                            w = w.__replace__(wait_value=nv)
                            changed = True
                    new_waits.append(w)
                new_upd = []
                for u in si.on_update:
                    if (u.id == s and u.update_mode in
                            ("sem-add-imm", "sem-sub-imm")
                            and u.update_value == total):
                        u = u.__replace__(update_value=len(keep))
                        changed = True
                    new_upd.append(u)
                if changed:
                    i.sync_info = si.__replace__(
                        on_wait=new_waits, on_update=new_upd)


def _copy(nc, engine, out, in_):
    import concourse.mybir as mybir
    if engine == "act":
        nc.scalar.activation(out=out, in_=in_,
                             func=mybir.ActivationFunctionType.Copy)
    else:
        nc.vector.tensor_copy(out=out, in_=in_)


def _pipelined_groups(nc, tc, mybir, n_groups, dma_b, cfg, env):
    f32 = mybir.dt.float32
    bf16 = mybir.dt.bfloat16
    x, out = env["x"], env["out"]
    wup2_sb, wdn3_sb = env["wup2_sb"], env["wdn3_sb"]
    wdn2_sb, alpha_sb = env["wdn2_sb"], env["alpha_sb"]
    wc2_sb = env["wc2_sb"]
    xpool, t1ps_pool, t1sb_pool = env["xpool"], env["t1ps_pool"], env["t1sb_pool"]
    ups_pool, ssb_pool, vps_pool = env["ups_pool"], env["ssb_pool"], env["vps_pool"]
    vsb_pool, yps_pool, ysb_pool = env["vsb_pool"], env["yps_pool"], env["ysb_pool"]

    abl = cfg["ABL"]
    cpool = env["cpool"]
    dummies = {}
    if "m1" in abl or "t1c" in abl:
        shp = [128, 2, 256] if cfg["M1_ROWTILE"] else [128, 512]
        d = cpool.tile(shp, bf16, name="dummy_t1sb")
        nc.vector.memset(d[:], 0.25)
        dummies["t1sb"] = d
    if "m2" in abl or "prelu" in abl:
        d = cpool.tile([128, 1024], bf16, name="dummy_ssb")
        nc.vector.memset(d[:], 0.25)
        dummies["ssb"] = d
    if "m3" in abl or "vc" in abl:
        d = cpool.tile([128, 512], bf16, name="dummy_vsb")
        nc.vector.memset(d[:], 0.25)
        dummies["vsb"] = d
    if "m4" in abl or "yc" in abl:
        d = cpool.tile([128, dma_b * 256], bf16, name="dummy_ysb")
        nc.vector.memset(d[:], 0.25)
        dummies["ysb"] = d
    state = {}          # per-group tiles passed between stages
    xt_cur = [None]
    ysb_cur = [None]
    yps_cur = [None]

    def stage_a(g):
        gb = g % dma_b
        if cfg["M1_ROWTILE"]:
            # xt [128, 256/group]: rows 0-63 = pairs {2s}, 64-127 = {2s+1}
            if gb == 0:
                xt_cur[0] = xpool.tile([128, dma_b * 256], bf16, name="xt4")
                nc.sync.dma_start(
                    out=xt_cur[0][:], in_=x[:, g * 256:(g + dma_b) * 256])
            xt = xt_cur[0][:, gb * 256:(gb + 1) * 256]
            # M1: 2 slots x 2 row-tiled concurrent MMs -> 2 PSUM banks
            # ([128, 2, 512] f32 spans 2 banks; only cols 0:256 used)
            t1ps = t1ps_pool.tile([128, 2, 512], f32)
            for s in ([] if "m1" in abl else range(2)):
                for rh in range(2):
                    nc.tensor.matmul(
                        t1ps[:, rh:rh + 1, s * 128:(s + 1) * 128],
                        lhsT=xt[rh * 64:(rh + 1) * 64,
                                s * 128:(s + 1) * 128],
                        rhs=wup2_sb[rh * 64:(rh + 1) * 64, :],
                        start=True, stop=True,
                    )
            if "t1c" not in abl:
                t1sb = t1sb_pool.tile([128, 2, 256], bf16)
                _copy(nc, cfg["EVAC_T1"], t1sb[:, :, :], t1ps[:, :, 0:256])
            else:
                t1sb = dummies["t1sb"]
        else:
            if gb == 0:
                xt_cur[0] = xpool.tile([64, dma_b * 512], bf16, name="xt4")
                nc.sync.dma_start(
                    out=xt_cur[0][:], in_=x[:, g * 512:(g + dma_b) * 512])
            xt = xt_cur[0][:, gb * 512:(gb + 1) * 512]

            # M1: 4 MMs (K=64) -> t1 [128 (e,w), 512 (p4,h')] one PSUM bank
            t1ps = t1ps_pool.tile([128, 512], f32)
            for p4 in ([] if "m1" in abl else range(4)):
                nc.tensor.matmul(
                    t1ps[:, p4 * 128:(p4 + 1) * 128],
                    lhsT=xt[:, p4 * 128:(p4 + 1) * 128],
                    rhs=wup2_sb[0:64, :],
                    start=True, stop=True,
                )
            if "t1c" not in abl:
                t1sb = t1sb_pool.tile([128, 512], bf16)
                _copy(nc, cfg["EVAC_T1"], t1sb[:], t1ps[:])
            else:
                t1sb = dummies["t1sb"]
        state[g] = {"t1sb": t1sb}


    def stage_b(g):
        st = state[g]
        t1sb = st["t1sb"]

        def t1rhs(e, half=None):
            if cfg["M1_ROWTILE"]:
                if half is None:
                    return t1sb[e * 64:(e + 1) * 64, :, :]
                return t1sb[e * 64:(e + 1) * 64, half:half + 1, :]
            if half is None:
                return t1sb[e * 64:(e + 1) * 64, :]
            return t1sb[e * 64:(e + 1) * 64, half * 256:(half + 1) * 256]

        # M2: 2 row-tiled MMs (K=64, N=512) -> u [128 w', (e,q,h')]
        ups = ups_pool.tile([128, 1024], f32)
        if "m2" in abl:
            # timing ablation: keep one cheap MM so ups has a writer
            # (the Tile allocator refuses read-only tiles)
            nc.tensor.matmul(
                ups[:, 0:512],
                lhsT=wup2_sb[0:64, :], rhs=t1rhs(0) if t1sb is not None
                else wup2_sb[0:64, 0:512],
                start=True, stop=True,
            )
        elif cfg["M2_ROWTILE"]:
            for e in range(2):
                nc.tensor.matmul(
                    ups[:, e * 512:(e + 1) * 512],
                    lhsT=wup2_sb[e * 64:(e + 1) * 64, :],
                    rhs=t1rhs(e),
                    start=True, stop=True,
                )
        else:
            for e in range(2):
                for half in range(2):
                    nc.tensor.matmul(
                        ups[:, e * 512 + half * 256:e * 512 + (half + 1) * 256],
                        lhsT=wup2_sb[e * 64:(e + 1) * 64, :],
                        rhs=t1rhs(e, half),
                        start=True, stop=True,
                    )
        if "prelu" in abl:
            st["ssb"] = dummies["ssb"]
            return
        ssb = ssb_pool.tile([128, 1024], bf16)
        if cfg["RELU_SPLIT"]:
            rc = cfg["RELU_ACT_COLS"]
            if rc > 0:
                nc.scalar.activation(
                    out=ssb[:, 0:rc], in_=ups[:, 0:rc],
                    func=mybir.ActivationFunctionType.Relu)
            if rc < 1024:
                nc.vector.tensor_scalar_max(ssb[:, rc:1024],
                                            ups[:, rc:1024], 0.0)
        elif cfg["PRELU_MODE"] == "gpsimd":
            # 2-step: evac u PSUM->SBUF bf16 (engine per UCOPY_*), then
            # lrelu on GPSIMD from SBUF: ssb = max(alpha*u_sb, u_sb).
            # Moves the nonlinearity to a third engine; ACT/DVE only copy.
            usb = ssb_pool.tile([128, 1024], bf16, name="usb", tag="usb")
            ca = cfg["UCOPY_ACT_COLS"]
            if ca > 0:
                nc.scalar.activation(
                    out=usb[:, 0:ca], in_=ups[:, 0:ca],
                    func=mybir.ActivationFunctionType.Copy)
            if ca < 1024:
                nc.vector.tensor_copy(out=usb[:, ca:1024],
                                      in_=ups[:, ca:1024])
            nc.gpsimd.scalar_tensor_tensor(
                out=ssb[:], in0=usb[:], scalar=NEG_SLOPE, in1=usb[:],
                op0=mybir.AluOpType.mult, op1=mybir.AluOpType.max)
        else:
            ac = cfg["PRELU_ACT_COLS"]
            if ac > 0:
                nc.scalar.activation(
                    out=ssb[:, 0:ac], in_=ups[:, 0:ac],
                    func=mybir.ActivationFunctionType.Prelu,
                    scale=1.0, alpha=alpha_sb[:],
                )
            if ac < 1024:
                scr = ssb_pool.tile([128, 1024 - ac], bf16, name="lrl_scr",
                                    tag="lrl_scr")
                nc.vector.tensor_scalar_mul(scr[:], ups[:, ac:1024],
                                            NEG_SLOPE)
                nc.vector.tensor_tensor(
                    out=ssb[:, ac:1024], in0=ups[:, ac:1024], in1=scr[:],
                    op=mybir.AluOpType.max)
        st["ssb"] = ssb

    def stage_c(g):
        st = state.pop(g)
        ssb = st["ssb"]
        t1sb_c = st.get("t1sb")
        gb = g % dma_b
        if "yc" in abl or "m4" in abl:
            ysb4 = dummies["ysb"]
        else:
            if gb == 0:
                ysb_cur[0] = ysb_pool.tile([128, dma_b * 256], bf16,
                                           name="ysb4")
            ysb4 = ysb_cur[0]

        # M3: 8 MMs -> v per plane [128 h', 64 w'']
        qpos = [0, 2, 1, 3] if cfg["M1_ROWTILE"] else [0, 1, 2, 3]
        vps = vps_pool.tile([128, 512], f32)
        if cfg["RELU_SPLIT"] and "m3" not in abl:
            # M3b: linear 0.2*u path straight from t1sb: per channel-pair k
            # out block [128 h', 128 (e,w'')] = t1_blk.T @ blockdiag(Wc, Wc)
            for k in range(4):
                q = qpos[k]
                if cfg["M1_ROWTILE"]:
                    blk = t1sb_c[:, q // 2, (q % 2) * 128:(q % 2 + 1) * 128]
                else:
                    blk = t1sb_c[:, q * 128:(q + 1) * 128]
                nc.tensor.matmul(
                    vps[:, k * 128:(k + 1) * 128],
                    lhsT=blk, rhs=wc2_sb[:],
                    start=(k == 0), stop=False, skip_group_check=True,
                )
            for p in range(GROUP):
                k, e = p // 2, p % 2
                s_off = e * 512 + qpos[k] * 128
                nc.tensor.matmul(
                    vps[:, p * 64:(p + 1) * 64],
                    lhsT=ssb[:, s_off:s_off + 128],
                    rhs=wdn3_sb[:],
                    start=False, stop=(p == GROUP - 1),
                    skip_group_check=True,
                )
        else:
            grp = cfg["MM_GROUP"]
            for p in ([] if "m3" in abl else range(GROUP)):
                p4, e = p // 2, p % 2
                s_off = e * 512 + qpos[p4] * 128
                nc.tensor.matmul(
                    vps[:, p * 64:(p + 1) * 64],
                    lhsT=ssb[:, s_off:s_off + 128],
                    rhs=wdn3_sb[:],
                    start=(p == 0) if grp else True,
                    stop=(p == GROUP - 1) if grp else True,
                    skip_group_check=grp,
                )
        if "vc" not in abl and "m3" not in abl:
            vsb = vsb_pool.tile([128, 512], bf16)
            _copy(nc, cfg["EVAC_V"], vsb[:], vps[:])
        else:
            vsb = dummies["vsb"]

        # M4: 2 col-tiled MMs -> y [128 (ph,h''), 256 (pl,w'')]
        if cfg["Y_BATCH"]:
            # yps batches 2 groups (1 PSUM bank); single evac instr per pair
            yb = g % 2
            if yb == 0:
                yps_cur[0] = yps_pool.tile([128, 512], f32, name="yps4")
            yps4 = yps_cur[0]
            for ph in ([] if "m4" in abl else range(2)):
                nc.tensor.matmul(
                    yps4[ph * 64:(ph + 1) * 64, yb * 256:(yb + 1) * 256],
                    lhsT=wdn2_sb[:, ph * 64:(ph + 1) * 64],
                    rhs=vsb[:, ph * 256:(ph + 1) * 256],
                    start=True, stop=True,
                )
            if yb == 1 and "yc" not in abl and "m4" not in abl:
                pair_off = (g - 1) % dma_b
                _copy(nc, cfg["EVAC_Y"],
                      ysb4[:, pair_off * 256:(pair_off + 2) * 256], yps4[:])
        elif cfg["Y_COLTILE"]:
            yps = yps_pool.tile([128, 256], f32)
            for ph in ([] if "m4" in abl else range(2)):
                nc.tensor.matmul(
                    yps[ph * 64:(ph + 1) * 64, :],
                    lhsT=wdn2_sb[:, ph * 64:(ph + 1) * 64],
                    rhs=vsb[:, ph * 256:(ph + 1) * 256],
                    start=True, stop=True,
                )
            if "yc" not in abl and "m4" not in abl:
                _copy(nc, cfg["EVAC_Y"], ysb4[:, gb * 256:(gb + 1) * 256], yps[:])
        else:
            yps = yps_pool.tile([64, 512], f32)
            nc.tensor.matmul(
                yps[:], lhsT=wdn2_sb[:, 0:64], rhs=vsb[:],
                start=True, stop=True,
            )
            _copy(nc, cfg["EVAC_Y"],
                  ysb4[0:64, gb * 256:(gb + 1) * 256], yps[:, 0:256])
            _copy(nc, cfg["EVAC_Y"],
                  ysb4[64:128, gb * 256:(gb + 1) * 256], yps[:, 256:512])

        if gb == dma_b - 1:
            nc.sync.dma_start(
                out=out[:, (g - dma_b + 1) * 256:(g + 1) * 256],
                in_=ysb4[:],
            )

    # skewed emission: B(g+1) before A(g+2) puts M2 ahead of the next M1
    # in the PE FIFO (M1 can stall on the t1 PSUM buffer; M2 feeds ACT)
    skew_b = cfg["SKEW_B"]
    skew_total = cfg["SKEW_C"]
    b_first = cfg["B_FIRST"]
    for i in range(n_groups + skew_total):
        ga = i
        gb_ = i - (skew_total - skew_b)
        gc = i - skew_total
        if b_first and 0 <= gb_ < n_groups:
            stage_b(gb_)
        if ga < n_groups:
            stage_a(ga)
        if not b_first and 0 <= gb_ < n_groups:
            stage_b(gb_)
        if 0 <= gc < n_groups:
            stage_c(gc)


def kernel(input, bias, up_filter, down_filter):
    global _LAST_RESULT, _CACHED
    from concourse.bass_utils import run_bass_kernel_spmd

    input = np.asarray(input, dtype=np.float32)
    bias = np.asarray(bias, dtype=np.float32)
    if np.any(bias):
        input = input + bias.reshape(1, C, 1, 1)

    if _CACHED is None:
        _CACHED = _build_bass()
    nc = _CACHED

    wup2_m, wdn3_m, wdn2_m, wc2_m = _build_consts(up_filter, down_filter)

    in_maps = []
    for i in range(N_CORES):
        in_maps.append({
            "x": _pack_x(input[i]),
            "wup2": wup2_m,
            "wdn3": wdn3_m,
            "wdn2": wdn2_m,
            "wc2": wc2_m,
        })

    res = run_bass_kernel_spmd(nc, in_maps, core_ids=list(range(N_CORES)))
    _LAST_RESULT = res
    y = np.stack([_unpack_y(r["out"]) for r in res.results], axis=0)
    return np.ascontiguousarray(y)

